# revision 8
# baseline (speedup 1.0000x reference)
"""Masked 5x5 group-causal conv (PixelCNN-style) + bias + per-channel PReLU.

Problem: x (8, 128, 256, 512) f32, weight (128, 128, 5, 5) f32 masked by a
fixed causal mask, SAME conv, + bias + PReLU.  The mask leaves 13 live taps:
ky=0,1 (dy=-2,-1): all 5 kx; ky=2 (dy=0): kx=0,1 and the group-masked
center tap (2,2).  Mask folded into weights on the host.

Sharding: data-parallel over batch - core i computes batch element i.

dr8 path (default): 2x tensor-engine throughput via fp8 DoubleRow pairs.
  - The 12 non-center taps run as 6 DoubleRow fp8 matmuls (two taps per
    matmul: one at 1 cyc/row does 2 taps' work).  fp8 e4m3 quantization
    error is halved by centering: w' = w - 0.5 (w ~ U[0,1]), and adding the
    mean part back exactly: 0.5 * A12(h,w), A12 = 12-tap spatial sum of
    S(h,w) = sum_cin x.  S rows are computed by per-row fp32r matmuls with
    indicator stationaries that accumulate each row at its own PSUM lane;
    A12 is assembled on the DVE (stream_shuffle for the +-1/2 row shifts,
    tensor_add for the dx shifts) and broadcast-added into each row's PSUM
    by a tiny selector matmul (contraction 8).
  - center tap (2,2) runs as a fp32r matmul on the raw f32 rows.
  - measured rel err vs f32 reference ~1.6e-2 (gate 2e-2); fp16 fallback
    path kept below (~3e-4).

fp16 path: 13 fp16 matmuls per row (the previous implementation).
"""

import numpy as np

B, C, H, W = 8, 128, 256, 512
KS = 5
PAD = 2
RB = 8  # rows per band
NBANDS = H // RB
WP = W + 2 * PAD
TR = RB + 2  # band tile rows: h0-2 .. h0+7

# 13 live taps (ky, kx) of the causal mask.
TAPS = [(ky, kx) for ky in range(2) for kx in range(KS)] + [(2, 0), (2, 1), (2, 2)]
NT = len(TAPS)

# dr8 pairing: 5 row-pairs (0,kx)+(1,kx), 1 self-pair (2,0)+(2,1)
PAIRS = [((0, kx), (1, kx)) for kx in range(KS)] + [((2, 0), (2, 1))]
TAPS12 = [t for p in PAIRS for t in p]

NGROUP, CIN_G, COUT_G = 8, 16, 16


def _build_mask() -> np.ndarray:
    c = KS // 2
    m = np.zeros((C, C, KS, KS), dtype=np.float32)
    m[:, :, :c, :] = 1.0
    m[:, :, c, :c] = 1.0
    g_out = np.arange(C)[:, None] // COUT_G
    g_in = np.arange(C)[None, :] // CIN_G
    m[:, :, c, c] = (g_in <= g_out).astype(np.float32)
    return m


_CACHE = {}


def _build_dr8():
    import concourse.bacc as bacc
    import concourse.mybir as mybir
    from concourse.tile import TileContext
    from concourse.ap import AP

    dt = mybir.dt
    F8 = dt.float8e4
    F16 = dt.float16
    F32R = dt.float32r
    DR = mybir.MatmulPerfMode.DoubleRow

    nc = bacc.Bacc("TRN2", target_bir_lowering=False)
    x = nc.dram_tensor("x", [C, H, W], F32R, kind="ExternalInput")
    w8 = nc.dram_tensor("w8", [C, 6, 2, C], F8, kind="ExternalInput")
    wc8 = nc.dram_tensor("wc8", [C, 2, C], F8, kind="ExternalInput")
    ind8 = nc.dram_tensor("ind8", [C, RB, 2, 16], F8, kind="ExternalInput")
    ind32 = nc.dram_tensor("ind32", [C, RB, 10], F32R, kind="ExternalInput")
    bias = nc.dram_tensor("bias", [C, 1], dt.float32, kind="ExternalInput")
    slope = nc.dram_tensor("slope", [C, 1], dt.float32, kind="ExternalInput")
    y = nc.dram_tensor("y", [C, H, W], dt.float32, kind="ExternalOutput")

    # S band lane layout (32, WP): lanes 0..7 = S rows h0..h0+7,
    # lanes 8,9 = halo rows h0-2, h0-1 (shuffled from the previous band).
    shift1 = [9] + [p - 1 for p in range(1, 32)]  # lane p <- S(h0+p-1)
    shift2 = [8, 9] + [p - 2 for p in range(2, 32)]  # lane p <- S(h0+p-2)
    halo = list(range(8)) + [6, 7] + list(range(10, 32))

    with TileContext(nc) as tc:
        with (
            tc.tile_pool(name="const", bufs=1) as cpool,
            tc.tile_pool(name="xin", bufs=3) as xin_pool,
            tc.tile_pool(name="a12", bufs=2) as a_pool,
            tc.tile_pool(name="oband", bufs=3) as out_pool,
            tc.tile_pool(name="ps", bufs=6, space="PSUM") as ps_pool,
            tc.tile_pool(name="pss", bufs=2, space="PSUM") as pss_pool,
        ):
            # --- startup DMAs -------------------------------------------------
            xin0 = xin_pool.tile([C, RB, W], F32R, name="xin")
            nc.sync.dma_start(xin0[:, 0:2, :], x[:, 0:2, :])
            ind_sb = cpool.tile([C, RB, 2, 16], F8, name="ind")
            nc.sync.dma_start(ind_sb[:, :, :, :], ind8[:, :, :, :])
            ind32_sb = cpool.tile([C, RB, 10], F32R, name="ind32")
            nc.sync.dma_start(ind32_sb[:, :, :], ind32[:, :, :])
            nc.sync.dma_start(xin0[:, 2:5, :], x[:, 2:5, :])
            w8_sb = cpool.tile([C, 6, 2, C], F8, name="w8")
            nc.sync.dma_start(w8_sb[:, :, :, :], w8[:, :, :, :])
            nc.sync.dma_start(xin0[:, 5:8, :], x[:, 5:8, :])
            wc_sb = cpool.tile([C, 2, C], F8, name="wc")
            nc.sync.dma_start(wc_sb[:, :, :], wc8[:, :, :])
            bias_sb = cpool.tile([C, 1], dt.float32, name="bias_sb")
            nc.sync.dma_start(bias_sb[:, :], bias[:, :])
            slope_sb = cpool.tile([C, 1], dt.float32, name="slope_sb")
            nc.sync.dma_start(slope_sb[:, :], slope[:, :])

            # persistent x8d ring: member 0 = e4m3(x), member 1 = e4m3 residual.
            # pads zeroed once here, never re-memset.
            x8r = [cpool.tile([C, TR, 2, WP], F8, name=f"x8r{i}") for i in range(4)]
            for t in x8r:
                nc.vector.memset(t[:, :, :, 0:PAD], 0.0)
                nc.vector.memset(t[:, :, :, W + PAD : WP], 0.0)
            sbr = [cpool.tile([32, WP], F16, name=f"sbr{i}") for i in range(2)]
            for t in sbr:
                nc.vector.memset(t[:, :], 0.0)

            # PE warm-up against the HAM clock gate while startup DMAs land.
            warm = cpool.tile([C, 2, C], F8, name="warm")
            nc.vector.memset(warm[:, :, :], 0.0)
            ps_warm = ps_pool.tile([C, W], dt.float32, name="psr")
            for _ in range(9):
                nc.tensor.matmul(
                    ps_warm[:, 0:C],
                    warm[:, :, :],
                    warm[:, 0:2, 0:C],
                    start=True,
                    stop=True,
                    perf_mode=DR,
                )

            xins = {}
            sbands = {}
            a12s = {}
            psum_s = {}

            def cast_chunk(b, j0, nr):
                # x8d row = 2 + xin row
                x8 = x8r[b % 4]
                xin = xins[b]
                r0 = 2 + j0
                nc.vector.tensor_copy(
                    x8[:, r0 : r0 + nr, 0, PAD : W + PAD],
                    xin[:, j0 : j0 + nr, :].bitcast(dt.float32),
                )
                nc.vector.tensor_sub(
                    x8[:, r0 : r0 + nr, 1, PAD : W + PAD],
                    xin[:, j0 : j0 + nr, :].bitcast(dt.float32),
                    x8[:, r0 : r0 + nr, 0, PAD : W + PAD],
                )

            def halo_copy(b):
                # x8d rows 0,1 (member 0) <- previous band tile rows 8,9
                nc.vector.tensor_copy(
                    x8r[b % 4][:, 0:2, 0, :], x8r[(b - 1) % 4][:, 8:10, 0, :]
                )

            # band 0: rows -2,-1 are zero
            xins[0] = xin0
            nc.vector.memset(x8r[0][:, 0:2, 0, PAD : W + PAD], 0.0)
            for j0, nr in ((0, 4), (4, 4)):
                cast_chunk(0, j0, nr)

            def s_sweep_row(b, r, f32=False):
                if r == 0:
                    psum_s[b] = pss_pool.tile([10, W], dt.float32, name="pss")
                if f32:
                    nc.tensor.matmul(
                        psum_s[b][:, :],
                        ind32_sb[:, r, :],
                        xins[b][:, r, :],
                        start=(r == 0),
                        stop=(r == RB - 1),
                    )
                else:
                    nc.tensor.matmul(
                        psum_s[b][:, :],
                        ind_sb[:, r, :, 0:10],
                        x8r[b % 4][:, 2 + r, :, PAD : W + PAD],
                        start=(r == 0),
                        stop=(r == RB - 1),
                        perf_mode=DR,
                    )

            def sband_drain(b):
                sb = sbr[b % 2]
                if b - 1 in sbands:
                    nc.vector.stream_shuffle(
                        sb[0:32, PAD : W + PAD],
                        sbands[b - 1][0:32, PAD : W + PAD],
                        halo,
                    )
                    nc.scalar.copy(sb[0:8, PAD : W + PAD], psum_s[b][0:8, :])
                else:
                    # band 0: psum lanes 8,9 hold accumulated zeros
                    nc.scalar.copy(sb[0:10, PAD : W + PAD], psum_s[b][0:10, :])
                sbands[b] = sb

            def a12_assemble(b):
                sb = sbands[b]
                s1 = a_pool.tile([32, WP], F16, name="s1", tag="s1")
                s2 = a_pool.tile([32, WP], F16, name="s2", tag="s2")
                nc.vector.stream_shuffle(s1[:, :], sb[:, :], shift1)
                nc.vector.stream_shuffle(s2[:, :], sb[:, :], shift2)
                u = a_pool.tile([8, WP], F16, name="u", tag="u")
                nc.vector.tensor_add(u[:, :], s1[0:8, :], s2[0:8, :])
                t1 = a_pool.tile([8, W], F16, name="t1", tag="t1")
                t2 = a_pool.tile([8, W], F16, name="t2", tag="t2")
                nc.vector.tensor_add(t1[:, :], u[:, 0:W], u[:, 1 : 1 + W])
                nc.vector.tensor_add(t2[:, :], u[:, 2 : 2 + W], u[:, 3 : 3 + W])
                nc.vector.tensor_add(t1[:, :], t1[:, :], t2[:, :])
                nc.vector.tensor_add(t1[:, :], t1[:, :], u[:, 4 : 4 + W])
                nc.vector.tensor_add(t2[:, :], sb[0:8, 0:W], sb[0:8, 1 : 1 + W])
                a = a_pool.tile([8, W], F16, name="a12", tag="a12")
                nc.vector.tensor_add(a[:, :], t1[:, :], t2[:, :])
                ad = a_pool.tile([8, 2, W], F8, name="a12d", tag="a12d")
                nc.scalar.copy(ad[:, 0, :], a[:, :])
                nc.vector.tensor_sub(ad[:, 1, :], a[:, :], ad[:, 0, :])
                # scatter A12 main/resid into the x8 tile's member-1 lanes
                # 0/1 (rows 2..9); the center-tap matmul picks them up via
                # wc8[0:2, 1, :] = 0.5 (one pass fewer per row).
                x8 = x8r[b % 4]
                nc.sync.dma_start(x8[0:1, 2:10, 1, PAD : W + PAD], ad[:, 0, :])
                nc.sync.dma_start(x8[1:2, 2:10, 1, PAD : W + PAD], ad[:, 1, :])

            def load_band_dma(b):
                h0 = b * RB
                xin = xin_pool.tile([C, RB, W], F32R, name="xin")
                for j0, nr in ((0, 2), (2, 2), (4, 2), (6, 2)):
                    nc.sync.dma_start(
                        xin[:, j0 : j0 + nr, :], x[:, h0 + j0 : h0 + j0 + nr, :]
                    )
                xins[b] = xin

            def cast_band(b):
                for j0, nr in ((0, 4), (4, 4)):
                    cast_chunk(b, j0, nr)
                halo_copy(b)

            # prologue: S sweeps for bands <= 2 read xin directly (fp32r) so
            # they are not gated on the fp8 casts; A12(0) is assembled as soon
            # as the band-0 sweeps finish.
            for r in range(RB):
                s_sweep_row(0, r, f32=True)
            sband_drain(0)
            a12_assemble(0)

            def load_prologue_band(bb):
                xinb_ = xin_pool.tile([C, RB, W], F32R, name="xin")
                for j0, nr in ((0, 2), (2, 2), (4, 2), (6, 2)):
                    nc.sync.dma_start(
                        xinb_[:, j0 : j0 + nr, :],
                        x[:, 8 * bb + j0 : 8 * bb + j0 + nr, :],
                    )
                xins[bb] = xinb_
                for j0, nr in ((0, 4), (4, 4)):
                    cast_chunk(bb, j0, nr)
                halo_copy(bb)

            load_prologue_band(1)
            for r in range(RB):
                s_sweep_row(1, r, f32=True)
            load_prologue_band(2)

            for b in range(NBANDS):
                if b + 3 < NBANDS:
                    load_band_dma(b + 3)
                if b + 1 < NBANDS:
                    sband_drain(b + 1)
                    a12_assemble(b + 1)
                if b + 3 < NBANDS:
                    cast_band(b + 3)
                x8b = x8r[b % 4]
                ob = out_pool.tile([C, RB, W], dt.float32, name="ob")
                for r in range(RB):
                    psr = ps_pool.tile([C, W], dt.float32, name="psr")
                    for k in range(6):
                        if k < 5:
                            rhs = x8b[:, r : r + 2, 0, k : k + W]
                        else:
                            base = x8b[:, 2 + r, 0, 0:W]
                            rhs = AP(
                                base.tensor,
                                base.offset,
                                [list(base.ap[0]), [1, 2], [1, W]],
                            )
                        nc.tensor.matmul(
                            psr[:, :],
                            w8_sb[:, k, :, :],
                            rhs,
                            start=(k == 0),
                            stop=False,
                            perf_mode=DR,
                        )
                    nc.tensor.matmul(
                        psr[:, :],
                        wc_sb[:, :, :],
                        x8b[:, 2 + r, :, PAD : W + PAD],
                        start=False,
                        stop=True,
                        perf_mode=DR,
                    )
                    if b + 2 < NBANDS:
                        s_sweep_row(b + 2, r)
                    nc.scalar.activation(
                        ob[:, r, :],
                        psr[:, :],
                        mybir.ActivationFunctionType.Prelu,
                        bias=bias_sb[:, 0:1],
                        scale=1.0,
                        alpha=slope_sb[:, 0:1],
                    )
                h0 = b * RB
                if b == NBANDS - 1:
                    for r0, nr in ((0, 2), (2, 2), (4, 1), (5, 1), (6, 1), (7, 1)):
                        nc.sync.dma_start(
                            y[:, h0 + r0 : h0 + r0 + nr, :], ob[:, r0 : r0 + nr, :]
                        )
                else:
                    nc.sync.dma_start(y[:, h0 : h0 + RB, :], ob[:, :, :])
                for d in (xins, sbands, a12s, psum_s):
                    d.pop(b - 1, None)
    nc.compile()
    return nc


def _prep_dr8_inputs(weight, bias, slope):
    import ml_dtypes

    f8 = ml_dtypes.float8_e4m3
    wm = np.asarray(weight, dtype=np.float32) * _build_mask()
    w8 = np.zeros((C, 6, 2, C), dtype=f8)
    for k, pair in enumerate(PAIRS):
        for j, (ky, kx) in enumerate(pair):
            w8[:, k, j, :] = (wm[:, :, ky, kx].T - 0.5).astype(f8)
    wc8 = np.zeros((C, 2, C), dtype=f8)
    wc8[:, 0, :] = wm[:, :, 2, 2].T.astype(f8)
    # member 1: lanes 0/1 of the x8 tile's member-1 slot hold A12
    # main/resid (injected with coeff 0.5); the remaining lanes still hold
    # x residuals, so keep the center tap's residual term for them.
    wc8[:, 1, :] = wc8[:, 0, :]
    wc8[0, 1, :] = 0.5
    wc8[1, 1, :] = 0.5
    ind8 = np.zeros((C, RB, 2, 16), dtype=f8)
    for r in range(RB):
        ind8[:, r, :, r] = 1.0
    ind32 = np.zeros((C, RB, 10), dtype=np.float32)
    for r in range(RB):
        ind32[:, r, r] = 1.0
    bias_in = np.ascontiguousarray(np.asarray(bias, dtype=np.float32).reshape(C, 1))
    slope_in = np.ascontiguousarray(np.asarray(slope, dtype=np.float32).reshape(C, 1))
    return {
        "w8": w8,
        "wc8": wc8,
        "ind8": ind8,
        "ind32": ind32,
        "bias": bias_in,
        "slope": slope_in,
    }


# --------------------------------------------------------------------------
# fp16 fallback path (previous implementation)
# --------------------------------------------------------------------------


def _build_bass(dtype_tag: str):
    import concourse.bacc as bacc
    import concourse.mybir as mybir
    from concourse.tile import TileContext

    dt = mybir.dt
    mm_dt = {"bf16": dt.bfloat16, "fp16": dt.float16}.get(dtype_tag, dt.float32r)

    nc = bacc.Bacc("TRN2", target_bir_lowering=False)
    x_dt = dt.float32r if dtype_tag == "fp32r" else dt.float32
    x = nc.dram_tensor("x", [C, H, W], x_dt, kind="ExternalInput")
    w = nc.dram_tensor("w", [C, NT * C], mm_dt, kind="ExternalInput")
    bias = nc.dram_tensor("bias", [C, 1], dt.float32, kind="ExternalInput")
    slope = nc.dram_tensor("slope", [C, 1], dt.float32, kind="ExternalInput")
    y = nc.dram_tensor("y", [C, H, W], dt.float32, kind="ExternalOutput")

    with TileContext(nc) as tc:
        with (
            tc.tile_pool(name="const", bufs=1) as cpool,
            tc.tile_pool(name="xin", bufs=3) as xin_pool,
            tc.tile_pool(name="xband", bufs=5) as xband_pool,
            tc.tile_pool(name="oband", bufs=4) as out_pool,
            tc.tile_pool(name="ps", bufs=8, space="PSUM") as psum_pool,
        ):
            warm = cpool.tile([C, W], mm_dt, name="warm")
            nc.gpsimd.memset(warm[:, :], 0.0)
            ps_warm = psum_pool.tile([C, W], dt.float32, name="ps")
            for _ in range(7):
                nc.tensor.matmul(
                    ps_warm[:, :], warm[:, 0:C], warm[:, :], start=True, stop=True
                )

            bands = {}

            def load_band(b, chunks=((0, RB // 2), (RB // 2, RB // 2))):
                h0 = b * RB
                xb = xband_pool.tile([C, RB, WP], mm_dt, name="xb")
                nc.gpsimd.memset(xb[:, :, 0:PAD], 0.0)
                nc.gpsimd.memset(xb[:, :, W + PAD : WP], 0.0)
                if dtype_tag != "fp32r":
                    xin = xin_pool.tile([C, RB, W], dt.float32, name="xin")
                    for r0, nr in chunks:
                        nc.sync.dma_start(
                            xin[:, r0 : r0 + nr, :], x[:, h0 + r0 : h0 + r0 + nr, :]
                        )
                        nc.vector.tensor_copy(
                            xb[:, r0 : r0 + nr, PAD : W + PAD], xin[:, r0 : r0 + nr, :]
                        )
                else:
                    for r0, nr in chunks:
                        nc.sync.dma_start(
                            xb[:, r0 : r0 + nr, PAD : W + PAD],
                            x[:, h0 + r0 : h0 + r0 + nr, :],
                        )
                bands[b] = xb

            xb0 = xband_pool.tile([C, RB, WP], mm_dt, name="xb")
            nc.gpsimd.memset(xb0[:, :, 0:PAD], 0.0)
            nc.gpsimd.memset(xb0[:, :, W + PAD : WP], 0.0)
            xin0 = xin_pool.tile([C, RB, W], dt.float32, name="xin")
            w_sb = cpool.tile([C, NT * C], mm_dt, name="w_sb")
            b0_chunks = [(0, 1), (1, 1), (2, 2), (4, 2), (6, 2)]
            for k, (r0, nr) in enumerate(b0_chunks):
                if dtype_tag != "fp32r":
                    nc.sync.dma_start(xin0[:, r0 : r0 + nr, :], x[:, r0 : r0 + nr, :])
                    nc.vector.tensor_copy(
                        xb0[:, r0 : r0 + nr, PAD : W + PAD], xin0[:, r0 : r0 + nr, :]
                    )
                else:
                    nc.sync.dma_start(
                        xb0[:, r0 : r0 + nr, PAD : W + PAD], x[:, r0 : r0 + nr, :]
                    )
                if k == 0:
                    nc.sync.dma_start(w_sb[:, 10 * C :], w[:, 10 * C :])
                elif k == 1:
                    nc.sync.dma_start(w_sb[:, : 10 * C], w[:, : 10 * C])
            bands[0] = xb0
            bias_sb = cpool.tile([C, 1], dt.float32, name="bias_sb")
            nc.sync.dma_start(bias_sb[:, :], bias[:, :])
            slope_sb = cpool.tile([C, 1], dt.float32, name="slope_sb")
            nc.sync.dma_start(slope_sb[:, :], slope[:, :])

            def row_ap(h, dx):
                b, r = divmod(h, RB)
                return bands[b][:, r, PAD + dx : PAD + dx + W]

            for b in range(NBANDS):
                if b + 1 < NBANDS:
                    load_band(b + 1)
                h0 = b * RB
                psums = [psum_pool.tile([C, W], dt.float32, name="ps") for _ in range(RB)]
                valid = []
                for r in range(RB):
                    h = h0 + r
                    valid.append(
                        [t for t, (ky, kx) in enumerate(TAPS) if h + ky - PAD >= 0]
                    )
                ob = out_pool.tile([C, RB, W], dt.float32, name="ob")
                for r in range(RB):
                    h = h0 + r
                    for t in valid[r]:
                        ky, kx = TAPS[t]
                        dy, dx = ky - PAD, kx - PAD
                        nc.tensor.matmul(
                            psums[r][:, :],
                            w_sb[:, t * C : (t + 1) * C],
                            row_ap(h + dy, dx),
                            start=(t == valid[r][0]),
                            stop=(t == valid[r][-1]),
                        )
                    nc.scalar.activation(
                        ob[:, r, :],
                        psums[r][:, :],
                        mybir.ActivationFunctionType.Prelu,
                        bias=bias_sb[:, 0:1],
                        scale=1.0,
                        alpha=slope_sb[:, 0:1],
                    )
                if b == NBANDS - 1:
                    for r0, nr in ((0, 2), (2, 2), (4, 1), (5, 1), (6, 1), (7, 1)):
                        nc.sync.dma_start(
                            y[:, h0 + r0 : h0 + r0 + nr, :], ob[:, r0 : r0 + nr, :]
                        )
                else:
                    nc.sync.dma_start(y[:, h0 : h0 + RB, :], ob[:, :, :])
                if b - 1 in bands:
                    del bands[b - 1]
    nc.compile()
    return nc


def _get_nc(dtype_tag: str):
    if dtype_tag not in _CACHE:
        if dtype_tag == "dr8":
            _CACHE[dtype_tag] = _build_dr8()
        else:
            _CACHE[dtype_tag] = _build_bass(dtype_tag)
    return _CACHE[dtype_tag]


def _prep_weights(weight: np.ndarray, dtype_tag: str) -> np.ndarray:
    wm = weight.astype(np.float32) * _build_mask()
    wt = np.transpose(wm, (2, 3, 1, 0))  # (ky, kx, cin, cout)
    w_taps = np.concatenate([wt[ky, kx] for ky, kx in TAPS], axis=1)
    if dtype_tag == "bf16":
        import ml_dtypes

        return np.ascontiguousarray(w_taps).astype(ml_dtypes.bfloat16)
    if dtype_tag == "fp16":
        return np.ascontiguousarray(w_taps).astype(np.float16)
    return np.ascontiguousarray(w_taps)


def kernel(x, weight, bias, slope, dtype_tag="dr8", trace=False):
    from concourse.bass_utils import run_bass_kernel_spmd

    nc = _get_nc(dtype_tag)
    x = np.asarray(x, dtype=np.float32)
    if dtype_tag == "dr8":
        consts = _prep_dr8_inputs(weight, bias, slope)
        in_maps = [{"x": np.ascontiguousarray(x[i]), **consts} for i in range(B)]
    else:
        w_in = _prep_weights(np.asarray(weight), dtype_tag)
        bias_in = np.ascontiguousarray(np.asarray(bias, dtype=np.float32).reshape(C, 1))
        slope_in = np.ascontiguousarray(
            np.asarray(slope, dtype=np.float32).reshape(C, 1)
        )
        in_maps = [
            {"x": np.ascontiguousarray(x[i]), "w": w_in, "bias": bias_in, "slope": slope_in}
            for i in range(B)
        ]
    res = run_bass_kernel_spmd(nc, in_maps, core_ids=list(range(B)), trace=trace)
    y = np.stack([res.results[i]["y"] for i in range(B)], axis=0)
    if trace:
        return y, res
    return y



# revision 9
# speedup vs baseline: 1.1983x; 1.1983x over previous
"""Masked 5x5 group-causal conv (PixelCNN-style) + bias + per-channel PReLU.

Problem: x (8, 128, 256, 512) f32, weight (128, 128, 5, 5) f32 masked by a
fixed causal mask, SAME conv, + bias + PReLU.  The mask leaves 13 live taps:
ky=0,1 (dy=-2,-1): all 5 kx; ky=2 (dy=0): kx=0,1 and the group-masked
center tap (2,2).  Mask folded into weights on the host.

Sharding: data-parallel over batch - core i computes batch element i.

dr8 path (default): 2x tensor-engine throughput via fp8 DoubleRow pairs.
  - The 12 non-center taps run as 6 DoubleRow fp8 matmuls (two taps per
    matmul: one at 1 cyc/row does 2 taps' work).  fp8 e4m3 quantization
    error is halved by centering: w' = w - 0.5 (w ~ U[0,1]), and adding the
    mean part back exactly: 0.5 * A12(h,w), A12 = 12-tap spatial sum of
    S(h,w) = sum_cin x.  S rows are computed by per-row fp32r matmuls with
    indicator stationaries that accumulate each row at its own PSUM lane;
    A12 is assembled on the DVE (stream_shuffle for the +-1/2 row shifts,
    tensor_add for the dx shifts) and broadcast-added into each row's PSUM
    by a tiny selector matmul (contraction 8).
  - center tap (2,2) runs as a fp32r matmul on the raw f32 rows.
  - measured rel err vs f32 reference ~1.6e-2 (gate 2e-2); fp16 fallback
    path kept below (~3e-4).

fp16 path: 13 fp16 matmuls per row (the previous implementation).
"""

import numpy as np

B, C, H, W = 8, 128, 256, 512
KS = 5
PAD = 2
RB = 8  # rows per band
NBANDS = H // RB
WP = W + 2 * PAD
TR = RB + 2  # band tile rows: h0-2 .. h0+7

# 13 live taps (ky, kx) of the causal mask.
TAPS = [(ky, kx) for ky in range(2) for kx in range(KS)] + [(2, 0), (2, 1), (2, 2)]
NT = len(TAPS)

# dr8 pairing: 5 row-pairs (0,kx)+(1,kx), 1 self-pair (2,0)+(2,1)
PAIRS = [((0, kx), (1, kx)) for kx in range(KS)] + [((2, 0), (2, 1))]
TAPS12 = [t for p in PAIRS for t in p]

NGROUP, CIN_G, COUT_G = 8, 16, 16


def _build_mask() -> np.ndarray:
    c = KS // 2
    m = np.zeros((C, C, KS, KS), dtype=np.float32)
    m[:, :, :c, :] = 1.0
    m[:, :, c, :c] = 1.0
    g_out = np.arange(C)[:, None] // COUT_G
    g_in = np.arange(C)[None, :] // CIN_G
    m[:, :, c, c] = (g_in <= g_out).astype(np.float32)
    return m


_CACHE = {}


def _build_dr8():
    import concourse.bacc as bacc
    import concourse.mybir as mybir
    from concourse.tile import TileContext
    from concourse.ap import AP

    dt = mybir.dt
    F8 = dt.float8e4
    F16 = dt.float16
    F32R = dt.float32r
    DR = mybir.MatmulPerfMode.DoubleRow

    nc = bacc.Bacc("TRN2", target_bir_lowering=False)
    x = nc.dram_tensor("x", [C, H, W], F32R, kind="ExternalInput")
    w8 = nc.dram_tensor("w8", [C, 6, 2, C], F8, kind="ExternalInput")
    wc8 = nc.dram_tensor("wc8", [C, 2, C], F8, kind="ExternalInput")
    ind8 = nc.dram_tensor("ind8", [C, RB, 2, 16], F8, kind="ExternalInput")
    ind32 = nc.dram_tensor("ind32", [C, RB, 10], F32R, kind="ExternalInput")
    bias = nc.dram_tensor("bias", [C, 1], dt.float32, kind="ExternalInput")
    slope = nc.dram_tensor("slope", [C, 1], dt.float32, kind="ExternalInput")
    y = nc.dram_tensor("y", [C, H, W], dt.float32, kind="ExternalOutput")

    # S band lane layout (32, WP): lanes 0..7 = S rows h0..h0+7,
    # lanes 8,9 = halo rows h0-2, h0-1 (shuffled from the previous band).
    shift1 = [9] + [p - 1 for p in range(1, 32)]  # lane p <- S(h0+p-1)
    shift2 = [8, 9] + [p - 2 for p in range(2, 32)]  # lane p <- S(h0+p-2)
    halo = list(range(8)) + [6, 7] + list(range(10, 32))

    with TileContext(nc) as tc:
        with (
            tc.tile_pool(name="const", bufs=1) as cpool,
            tc.tile_pool(name="xin", bufs=3) as xin_pool,
            tc.tile_pool(name="a12", bufs=2) as a_pool,
            tc.tile_pool(name="oband", bufs=3) as out_pool,
            tc.tile_pool(name="ps", bufs=6, space="PSUM") as ps_pool,
            tc.tile_pool(name="pss", bufs=2, space="PSUM") as pss_pool,
        ):
            # --- startup DMAs -------------------------------------------------
            xin0 = xin_pool.tile([C, RB, W], F32R, name="xin")
            nc.sync.dma_start(xin0[:, 0:2, :], x[:, 0:2, :])
            ind_sb = cpool.tile([C, RB, 2, 16], F8, name="ind")
            nc.sync.dma_start(ind_sb[:, :, :, :], ind8[:, :, :, :])
            ind32_sb = cpool.tile([C, RB, 10], F32R, name="ind32")
            nc.sync.dma_start(ind32_sb[:, :, :], ind32[:, :, :])
            nc.sync.dma_start(xin0[:, 2:5, :], x[:, 2:5, :])
            w8_sb = cpool.tile([C, 6, 2, C], F8, name="w8")
            nc.sync.dma_start(w8_sb[:, :, :, :], w8[:, :, :, :])
            nc.sync.dma_start(xin0[:, 5:8, :], x[:, 5:8, :])
            wc_sb = cpool.tile([C, 2, C], F8, name="wc")
            nc.sync.dma_start(wc_sb[:, :, :], wc8[:, :, :])
            bias_sb = cpool.tile([C, 1], dt.float32, name="bias_sb")
            nc.sync.dma_start(bias_sb[:, :], bias[:, :])
            slope_sb = cpool.tile([C, 1], dt.float32, name="slope_sb")
            nc.sync.dma_start(slope_sb[:, :], slope[:, :])

            # persistent x8d ring: member 0 = e4m3(x), member 1 = e4m3 residual.
            # pads zeroed once here, never re-memset.
            x8r = [cpool.tile([C, TR, 2, WP], F8, name=f"x8r{i}") for i in range(4)]
            for t in x8r:
                nc.vector.memset(t[:, :, :, 0:PAD], 0.0)
                nc.vector.memset(t[:, :, :, W + PAD : WP], 0.0)
            sbr = [cpool.tile([32, WP], F16, name=f"sbr{i}") for i in range(2)]
            for t in sbr:
                nc.vector.memset(t[:, :], 0.0)

            # PE warm-up against the HAM clock gate while startup DMAs land.
            warm = cpool.tile([C, 2, C], F8, name="warm")
            nc.vector.memset(warm[:, :, :], 0.0)
            ps_warm = ps_pool.tile([C, W], dt.float32, name="psr")
            for _ in range(9):
                nc.tensor.matmul(
                    ps_warm[:, 0:C],
                    warm[:, :, :],
                    warm[:, 0:2, 0:C],
                    start=True,
                    stop=True,
                    perf_mode=DR,
                )

            xins = {}
            sbands = {}
            a12s = {}
            psum_s = {}

            def cast_chunk(b, j0, nr):
                # x8d row = 2 + xin row
                x8 = x8r[b % 4]
                xin = xins[b]
                r0 = 2 + j0
                nc.vector.tensor_copy(
                    x8[:, r0 : r0 + nr, 0, PAD : W + PAD],
                    xin[:, j0 : j0 + nr, :].bitcast(dt.float32),
                )
                nc.vector.tensor_sub(
                    x8[:, r0 : r0 + nr, 1, PAD : W + PAD],
                    xin[:, j0 : j0 + nr, :].bitcast(dt.float32),
                    x8[:, r0 : r0 + nr, 0, PAD : W + PAD],
                )

            def halo_copy(b):
                # x8d rows 0,1 (member 0) <- previous band tile rows 8,9
                nc.vector.tensor_copy(
                    x8r[b % 4][:, 0:2, 0, :], x8r[(b - 1) % 4][:, 8:10, 0, :]
                )

            # band 0: rows -2,-1 are zero
            xins[0] = xin0
            nc.vector.memset(x8r[0][:, 0:2, 0, PAD : W + PAD], 0.0)
            for j0, nr in ((0, 4), (4, 4)):
                cast_chunk(0, j0, nr)

            def s_sweep_row(b, r, f32=False):
                if r == 0:
                    psum_s[b] = pss_pool.tile([10, W], dt.float32, name="pss")
                if f32:
                    nc.tensor.matmul(
                        psum_s[b][:, :],
                        ind32_sb[:, r, :],
                        xins[b][:, r, :],
                        start=(r == 0),
                        stop=(r == RB - 1),
                    )
                else:
                    nc.tensor.matmul(
                        psum_s[b][:, :],
                        ind_sb[:, r, :, 0:10],
                        x8r[b % 4][:, 2 + r, :, PAD : W + PAD],
                        start=(r == 0),
                        stop=(r == RB - 1),
                        perf_mode=DR,
                    )

            def sband_drain(b):
                sb = sbr[b % 2]
                if b - 1 in sbands:
                    nc.vector.stream_shuffle(
                        sb[0:32, PAD : W + PAD],
                        sbands[b - 1][0:32, PAD : W + PAD],
                        halo,
                    )
                    nc.scalar.copy(sb[0:8, PAD : W + PAD], psum_s[b][0:8, :])
                else:
                    # band 0: psum lanes 8,9 hold accumulated zeros
                    nc.scalar.copy(sb[0:10, PAD : W + PAD], psum_s[b][0:10, :])
                sbands[b] = sb

            def a12_assemble(b):
                sb = sbands[b]
                s1 = a_pool.tile([32, WP], F16, name="s1", tag="s1")
                s2 = a_pool.tile([32, WP], F16, name="s2", tag="s2")
                nc.vector.stream_shuffle(s1[:, :], sb[:, :], shift1)
                nc.vector.stream_shuffle(s2[:, :], sb[:, :], shift2)
                u = a_pool.tile([8, WP], F16, name="u", tag="u")
                nc.vector.tensor_add(u[:, :], s1[0:8, :], s2[0:8, :])
                t1 = a_pool.tile([8, W], F16, name="t1", tag="t1")
                t2 = a_pool.tile([8, W], F16, name="t2", tag="t2")
                nc.vector.tensor_add(t1[:, :], u[:, 0:W], u[:, 1 : 1 + W])
                nc.vector.tensor_add(t2[:, :], u[:, 2 : 2 + W], u[:, 3 : 3 + W])
                nc.vector.tensor_add(t1[:, :], t1[:, :], t2[:, :])
                nc.vector.tensor_add(t1[:, :], t1[:, :], u[:, 4 : 4 + W])
                nc.vector.tensor_add(t2[:, :], sb[0:8, 0:W], sb[0:8, 1 : 1 + W])
                a = a_pool.tile([8, W], F16, name="a12", tag="a12")
                nc.vector.tensor_add(a[:, :], t1[:, :], t2[:, :])
                ad = a_pool.tile([8, 2, W], F8, name="a12d", tag="a12d")
                nc.scalar.copy(ad[:, 0, :], a[:, :])
                nc.vector.tensor_sub(ad[:, 1, :], a[:, :], ad[:, 0, :])
                # scatter A12 main/resid into the x8 tile's member-1 lanes
                # 0/1 (rows 2..9); the center-tap matmul picks them up via
                # wc8[0:2, 1, :] = 0.5 (one pass fewer per row).
                x8 = x8r[b % 4]
                nc.sync.dma_start(x8[0:1, 2:10, 1, PAD : W + PAD], ad[:, 0, :])
                nc.sync.dma_start(x8[1:2, 2:10, 1, PAD : W + PAD], ad[:, 1, :])

            def load_band_dma(b):
                h0 = b * RB
                xin = xin_pool.tile([C, RB, W], F32R, name="xin")
                for j0, nr in ((0, 2), (2, 2), (4, 2), (6, 2)):
                    nc.sync.dma_start(
                        xin[:, j0 : j0 + nr, :], x[:, h0 + j0 : h0 + j0 + nr, :]
                    )
                xins[b] = xin

            def cast_band(b):
                for j0, nr in ((0, 4), (4, 4)):
                    cast_chunk(b, j0, nr)
                halo_copy(b)

            # prologue: S sweeps for bands <= 2 read xin directly (fp32r) so
            # they are not gated on the fp8 casts; A12(0) is assembled as soon
            # as the band-0 sweeps finish.
            for r in range(RB):
                s_sweep_row(0, r, f32=True)
            sband_drain(0)
            a12_assemble(0)

            def load_prologue_band(bb):
                xinb_ = xin_pool.tile([C, RB, W], F32R, name="xin")
                for j0, nr in ((0, 2), (2, 2), (4, 2), (6, 2)):
                    nc.sync.dma_start(
                        xinb_[:, j0 : j0 + nr, :],
                        x[:, 8 * bb + j0 : 8 * bb + j0 + nr, :],
                    )
                xins[bb] = xinb_
                for j0, nr in ((0, 4), (4, 4)):
                    cast_chunk(bb, j0, nr)
                halo_copy(bb)

            load_prologue_band(1)
            for r in range(RB):
                s_sweep_row(1, r, f32=True)
            load_prologue_band(2)

            for b in range(NBANDS):
                if b + 3 < NBANDS:
                    load_band_dma(b + 3)
                if b + 1 < NBANDS:
                    sband_drain(b + 1)
                    a12_assemble(b + 1)
                if b + 3 < NBANDS:
                    cast_band(b + 3)
                x8b = x8r[b % 4]
                ob = out_pool.tile([C, RB, W], dt.float32, name="ob")
                for r in range(RB):
                    psr = ps_pool.tile([C, W], dt.float32, name="psr")
                    for k in range(6):
                        if k < 5:
                            rhs = x8b[:, r : r + 2, 0, k : k + W]
                        else:
                            base = x8b[:, 2 + r, 0, 0:W]
                            rhs = AP(
                                base.tensor,
                                base.offset,
                                [list(base.ap[0]), [1, 2], [1, W]],
                            )
                        nc.tensor.matmul(
                            psr[:, :],
                            w8_sb[:, k, :, :],
                            rhs,
                            start=(k == 0),
                            stop=False,
                            perf_mode=DR,
                        )
                    nc.tensor.matmul(
                        psr[:, :],
                        wc_sb[:, :, :],
                        x8b[:, 2 + r, :, PAD : W + PAD],
                        start=False,
                        stop=True,
                        perf_mode=DR,
                    )
                    if b + 2 < NBANDS:
                        s_sweep_row(b + 2, r)
                    nc.scalar.activation(
                        ob[:, r, :],
                        psr[:, :],
                        mybir.ActivationFunctionType.Prelu,
                        bias=bias_sb[:, 0:1],
                        scale=1.0,
                        alpha=slope_sb[:, 0:1],
                    )
                h0 = b * RB
                if b == NBANDS - 1:
                    for r0, nr in ((0, 2), (2, 2), (4, 1), (5, 1), (6, 1), (7, 1)):
                        nc.sync.dma_start(
                            y[:, h0 + r0 : h0 + r0 + nr, :], ob[:, r0 : r0 + nr, :]
                        )
                else:
                    nc.sync.dma_start(y[:, h0 : h0 + RB, :], ob[:, :, :])
                for d in (xins, sbands, a12s, psum_s):
                    d.pop(b - 1, None)
    nc.compile()
    return nc


def _prep_dr8_inputs(weight, bias, slope):
    import ml_dtypes

    f8 = ml_dtypes.float8_e4m3
    wm = np.asarray(weight, dtype=np.float32) * _build_mask()
    w8 = np.zeros((C, 6, 2, C), dtype=f8)
    for k, pair in enumerate(PAIRS):
        for j, (ky, kx) in enumerate(pair):
            w8[:, k, j, :] = (wm[:, :, ky, kx].T - 0.5).astype(f8)
    wc8 = np.zeros((C, 2, C), dtype=f8)
    wc8[:, 0, :] = wm[:, :, 2, 2].T.astype(f8)
    # member 1: lanes 0/1 of the x8 tile's member-1 slot hold A12
    # main/resid (injected with coeff 0.5); the remaining lanes still hold
    # x residuals, so keep the center tap's residual term for them.
    # Rounded to {0, 0.5, 1}: the x-residual moving data is tiny, so the
    # extra quantization is negligible, and mantissa-free weights keep the
    # PE's switching activity (and DVFS throttle) low.
    wc8[:, 1, :] = (np.round(wm[:, :, 2, 2].T * 2.0) / 2.0).astype(f8)
    wc8[0, 1, :] = 0.5
    wc8[1, 1, :] = 0.5
    ind8 = np.zeros((C, RB, 2, 16), dtype=f8)
    for r in range(RB):
        ind8[:, r, :, r] = 1.0
    ind32 = np.zeros((C, RB, 10), dtype=np.float32)
    for r in range(RB):
        ind32[:, r, r] = 1.0
    bias_in = np.ascontiguousarray(np.asarray(bias, dtype=np.float32).reshape(C, 1))
    slope_in = np.ascontiguousarray(np.asarray(slope, dtype=np.float32).reshape(C, 1))
    return {
        "w8": w8,
        "wc8": wc8,
        "ind8": ind8,
        "ind32": ind32,
        "bias": bias_in,
        "slope": slope_in,
    }


# --------------------------------------------------------------------------
# fp16 fallback path (previous implementation)
# --------------------------------------------------------------------------


def _build_bass(dtype_tag: str):
    import concourse.bacc as bacc
    import concourse.mybir as mybir
    from concourse.tile import TileContext

    dt = mybir.dt
    mm_dt = {"bf16": dt.bfloat16, "fp16": dt.float16}.get(dtype_tag, dt.float32r)

    nc = bacc.Bacc("TRN2", target_bir_lowering=False)
    x_dt = dt.float32r if dtype_tag == "fp32r" else dt.float32
    x = nc.dram_tensor("x", [C, H, W], x_dt, kind="ExternalInput")
    w = nc.dram_tensor("w", [C, NT * C], mm_dt, kind="ExternalInput")
    bias = nc.dram_tensor("bias", [C, 1], dt.float32, kind="ExternalInput")
    slope = nc.dram_tensor("slope", [C, 1], dt.float32, kind="ExternalInput")
    y = nc.dram_tensor("y", [C, H, W], dt.float32, kind="ExternalOutput")

    with TileContext(nc) as tc:
        with (
            tc.tile_pool(name="const", bufs=1) as cpool,
            tc.tile_pool(name="xin", bufs=3) as xin_pool,
            tc.tile_pool(name="xband", bufs=5) as xband_pool,
            tc.tile_pool(name="oband", bufs=4) as out_pool,
            tc.tile_pool(name="ps", bufs=8, space="PSUM") as psum_pool,
        ):
            warm = cpool.tile([C, W], mm_dt, name="warm")
            nc.gpsimd.memset(warm[:, :], 0.0)
            ps_warm = psum_pool.tile([C, W], dt.float32, name="ps")
            for _ in range(7):
                nc.tensor.matmul(
                    ps_warm[:, :], warm[:, 0:C], warm[:, :], start=True, stop=True
                )

            bands = {}

            def load_band(b, chunks=((0, RB // 2), (RB // 2, RB // 2))):
                h0 = b * RB
                xb = xband_pool.tile([C, RB, WP], mm_dt, name="xb")
                nc.gpsimd.memset(xb[:, :, 0:PAD], 0.0)
                nc.gpsimd.memset(xb[:, :, W + PAD : WP], 0.0)
                if dtype_tag != "fp32r":
                    xin = xin_pool.tile([C, RB, W], dt.float32, name="xin")
                    for r0, nr in chunks:
                        nc.sync.dma_start(
                            xin[:, r0 : r0 + nr, :], x[:, h0 + r0 : h0 + r0 + nr, :]
                        )
                        nc.vector.tensor_copy(
                            xb[:, r0 : r0 + nr, PAD : W + PAD], xin[:, r0 : r0 + nr, :]
                        )
                else:
                    for r0, nr in chunks:
                        nc.sync.dma_start(
                            xb[:, r0 : r0 + nr, PAD : W + PAD],
                            x[:, h0 + r0 : h0 + r0 + nr, :],
                        )
                bands[b] = xb

            xb0 = xband_pool.tile([C, RB, WP], mm_dt, name="xb")
            nc.gpsimd.memset(xb0[:, :, 0:PAD], 0.0)
            nc.gpsimd.memset(xb0[:, :, W + PAD : WP], 0.0)
            xin0 = xin_pool.tile([C, RB, W], dt.float32, name="xin")
            w_sb = cpool.tile([C, NT * C], mm_dt, name="w_sb")
            b0_chunks = [(0, 1), (1, 1), (2, 2), (4, 2), (6, 2)]
            for k, (r0, nr) in enumerate(b0_chunks):
                if dtype_tag != "fp32r":
                    nc.sync.dma_start(xin0[:, r0 : r0 + nr, :], x[:, r0 : r0 + nr, :])
                    nc.vector.tensor_copy(
                        xb0[:, r0 : r0 + nr, PAD : W + PAD], xin0[:, r0 : r0 + nr, :]
                    )
                else:
                    nc.sync.dma_start(
                        xb0[:, r0 : r0 + nr, PAD : W + PAD], x[:, r0 : r0 + nr, :]
                    )
                if k == 0:
                    nc.sync.dma_start(w_sb[:, 10 * C :], w[:, 10 * C :])
                elif k == 1:
                    nc.sync.dma_start(w_sb[:, : 10 * C], w[:, : 10 * C])
            bands[0] = xb0
            bias_sb = cpool.tile([C, 1], dt.float32, name="bias_sb")
            nc.sync.dma_start(bias_sb[:, :], bias[:, :])
            slope_sb = cpool.tile([C, 1], dt.float32, name="slope_sb")
            nc.sync.dma_start(slope_sb[:, :], slope[:, :])

            def row_ap(h, dx):
                b, r = divmod(h, RB)
                return bands[b][:, r, PAD + dx : PAD + dx + W]

            for b in range(NBANDS):
                if b + 1 < NBANDS:
                    load_band(b + 1)
                h0 = b * RB
                psums = [psum_pool.tile([C, W], dt.float32, name="ps") for _ in range(RB)]
                valid = []
                for r in range(RB):
                    h = h0 + r
                    valid.append(
                        [t for t, (ky, kx) in enumerate(TAPS) if h + ky - PAD >= 0]
                    )
                ob = out_pool.tile([C, RB, W], dt.float32, name="ob")
                for r in range(RB):
                    h = h0 + r
                    for t in valid[r]:
                        ky, kx = TAPS[t]
                        dy, dx = ky - PAD, kx - PAD
                        nc.tensor.matmul(
                            psums[r][:, :],
                            w_sb[:, t * C : (t + 1) * C],
                            row_ap(h + dy, dx),
                            start=(t == valid[r][0]),
                            stop=(t == valid[r][-1]),
                        )
                    nc.scalar.activation(
                        ob[:, r, :],
                        psums[r][:, :],
                        mybir.ActivationFunctionType.Prelu,
                        bias=bias_sb[:, 0:1],
                        scale=1.0,
                        alpha=slope_sb[:, 0:1],
                    )
                if b == NBANDS - 1:
                    for r0, nr in ((0, 2), (2, 2), (4, 1), (5, 1), (6, 1), (7, 1)):
                        nc.sync.dma_start(
                            y[:, h0 + r0 : h0 + r0 + nr, :], ob[:, r0 : r0 + nr, :]
                        )
                else:
                    nc.sync.dma_start(y[:, h0 : h0 + RB, :], ob[:, :, :])
                if b - 1 in bands:
                    del bands[b - 1]
    nc.compile()
    return nc


def _get_nc(dtype_tag: str):
    if dtype_tag not in _CACHE:
        if dtype_tag == "dr8":
            _CACHE[dtype_tag] = _build_dr8()
        else:
            _CACHE[dtype_tag] = _build_bass(dtype_tag)
    return _CACHE[dtype_tag]


def _prep_weights(weight: np.ndarray, dtype_tag: str) -> np.ndarray:
    wm = weight.astype(np.float32) * _build_mask()
    wt = np.transpose(wm, (2, 3, 1, 0))  # (ky, kx, cin, cout)
    w_taps = np.concatenate([wt[ky, kx] for ky, kx in TAPS], axis=1)
    if dtype_tag == "bf16":
        import ml_dtypes

        return np.ascontiguousarray(w_taps).astype(ml_dtypes.bfloat16)
    if dtype_tag == "fp16":
        return np.ascontiguousarray(w_taps).astype(np.float16)
    return np.ascontiguousarray(w_taps)


def kernel(x, weight, bias, slope, dtype_tag="dr8", trace=False):
    from concourse.bass_utils import run_bass_kernel_spmd

    nc = _get_nc(dtype_tag)
    x = np.asarray(x, dtype=np.float32)
    if dtype_tag == "dr8":
        consts = _prep_dr8_inputs(weight, bias, slope)
        in_maps = [{"x": np.ascontiguousarray(x[i]), **consts} for i in range(B)]
    else:
        w_in = _prep_weights(np.asarray(weight), dtype_tag)
        bias_in = np.ascontiguousarray(np.asarray(bias, dtype=np.float32).reshape(C, 1))
        slope_in = np.ascontiguousarray(
            np.asarray(slope, dtype=np.float32).reshape(C, 1)
        )
        in_maps = [
            {"x": np.ascontiguousarray(x[i]), "w": w_in, "bias": bias_in, "slope": slope_in}
            for i in range(B)
        ]
    res = run_bass_kernel_spmd(nc, in_maps, core_ids=list(range(B)), trace=trace)
    y = np.stack([res.results[i]["y"] for i in range(B)], axis=0)
    if trace:
        return y, res
    return y

